# revision 79
# baseline (speedup 1.0000x reference)
"""Trainium2 Bass kernel for causal GQA self-attention (fused QKV + RoPE).

Problem: B=2, T=2048, C=2048, H=16 q-heads, KV=4 kv-heads, HD=128.
Sharding: 8 cores = (batch b, kv-group k). Each core computes the 4 q-heads
of one kv group for one batch element; outputs are disjoint slices of y.

v4 design (fp16 end-to-end, ~172us vs 177us v3 baseline; PE matmul floor
~147us):
  - All SBUF data fp16 (1 cycle/row on the PE at any width, half the DMA,
    2x DVE tensor_tensor mode). PSUM stays fp32.
  - qkv^T lives in 24 separate [128,512] tiles (per j-block x t-block) so
    every producer/consumer dependency is tile-exact (no false stalls).
  - tt=0-first schedule: all six j-blocks of t-block 0 project first
    (six-way interleaved across 6 PSUM accumulators so each arriving
    128KB x piece feeds 6 matmuls, matching startup DMA delivery), then
    attention passes start; the remaining 18 proj tiles drain inside the
    attention passes at deadline-driven, front-loaded budgets.
  - 14 dummy warmup matmuls burn the first-DMA wait and bring the PE out
    of its low p-state (silicon ramps ~2.7x->1x cycle time over ~3us of
    continuous execution) before real work lands.
  - DMA issue split across both HWDGE engines (x stream on nc.scalar,
    weights/consts/outputs on nc.sync), ordered by consumption time;
    w is cq-major so one DMA delivers all six j-blocks' weights per
    contraction quarter; bulk x[2]/x[3] issue from inside the schedule
    to keep startup HBM bandwidth for the critical window.  Only ~2KB
    contiguous per partition per transfer (128 descriptors/DMA).
  - RoPE partner swap is a DVE stream_shuffle (host perm places even/odd
    head-dim components 16 partitions apart within each 32-partition
    quadrant) -- no SBUF->SBUF DMAs; initial-block PSUM copies on Act.
  - Attention processes head pairs with two-step score lookahead:
    PSUM = 2x1 banks proj + 2x2 banks scores + 2 banks y = 8.
    Pass order (0,0),(0,2),...,(2,2),(3,2),(3,0) so the last pass's q
    tiles rope during the long (3,2) pass.  Within each drain group the
    v tile goes last (its trailing op is PE-side vtrans, not a DVE rope
    chain), giving rope-critical q tiles extra DVE slack.
  - Row sums of exp accumulate on DVE (fp16 tensor_tensor); the
    128-partition reduce + divide happen on host.
  - V transposed to s-major by PE matmul against identity; y stored as
    [tb, d, pair, head, t] so each pass's y is one 128-descriptor DMA;
    tail casts split so only a 128-col sliver trails the last matmul.
Output per core: unnormalized y^T [tb, 128, 2, 2, 512] fp16 + exp-sum
tiles [8, 128, 1024] fp16; host reduces, divides, transposes, concats.
"""

import math

import numpy as np

import concourse.bass as bass
import concourse.mybir as mybir
import concourse.tile as tile
from concourse import bacc
from concourse.bass_utils import run_bass_kernel_spmd

B, T, C = 2, 2048, 2048
H, KV, HD = 16, 4, 128
NREP = H // KV  # q heads per core
P = 128
NCORES = 8
CC = C // P  # 16 contraction chunks
TT = 4  # t-blocks of 512
TB = T // TT  # 512
SCALE = 1.0 / math.sqrt(HD)

f16 = mybir.dt.float16
f32 = mybir.dt.float32

TRACE = False  # set True (with ntff shim installed) to get exec_time_ns

_cache = {}


def _build():
    if "nc" in _cache:
        return _cache["nc"]

    nc = bacc.Bacc("TRN2", target_bir_lowering=False, debug=False,
                   num_devices=NCORES)

    # DRAM inputs (pre-laid-out on host for contiguous per-partition DMA)
    xT_d = nc.dram_tensor("xT", [TT, P, 4, 4, TB], f16, kind="ExternalInput").ap()
    # w in cq-major layout [cq, p, slot, ci, col], slot order [k,v,q0..q3]:
    # one DMA per cq delivers all six j-blocks' weights for that
    # contraction quarter, so the six-way interleaved startup projection
    # can consume each arriving x piece with 6 matmuls.
    wT_d = nc.dram_tensor("wT", [4, P, 6, 4, P], f16, kind="ExternalInput").ap()
    cc_d = nc.dram_tensor("CC", [P, T], f16, kind="ExternalInput").ap()
    ss_d = nc.dram_tensor("SS2", [P, T], f16, kind="ExternalInput").ap()
    tri_d = nc.dram_tensor("tri", [P, P], f16, kind="ExternalInput").ap()
    id_d = nc.dram_tensor("ident", [P, P], f16, kind="ExternalInput").ap()
    # outputs: y^T d-major [tb, d, head-pair, head, t] (2KB contiguous per
    # partition per pass: one 128-descriptor DMA per attention pass),
    # exp-sums per (tb, pass)
    yT_d = nc.dram_tensor("yT", [TT, P, 2, 2, TB], f16,
                          kind="ExternalOutput").ap()
    acc_d = nc.dram_tensor("acc", [TT * 2, P, 2 * TB], f16,
                           kind="ExternalOutput").ap()

    mult = mybir.AluOpType.mult
    add = mybir.AluOpType.add

    with tile.TileContext(nc) as tc:
        with (
            tc.tile_pool(name="big", bufs=1) as big_pool,
            tc.tile_pool(name="swp", bufs=2) as swp_pool,
            tc.tile_pool(name="ropetmp", bufs=2) as rt_pool,
            tc.tile_pool(name="expt", bufs=4) as exp_pool,
            tc.tile_pool(name="accp", bufs=2) as acc_pool,
            tc.tile_pool(name="yout", bufs=2) as y_pool,
            tc.tile_pool(name="warm", bufs=1) as warm_pool,
            tc.tile_pool(name="pp", bufs=2, space="PSUM") as pp_pool,
            tc.tile_pool(name="sp", bufs=2, space="PSUM") as sp_pool,
            tc.tile_pool(name="yp", bufs=1, space="PSUM") as yp_pool,
        ):
            # ---- resident tensors ----
            # w_sb [p, cq, slot, ci, col]; slot order [k, v, q0, q1, q2, q3]
            JS = {4: 0, 5: 1, 0: 2, 1: 3, 2: 4, 3: 5}
            w_sb = big_pool.tile([P, 4, 6, 4, P], f16, tag="w")
            x_sb = big_pool.tile([P, TT, 4, 4, TB], f16, tag="x")
            # qkv^T as separate tiles per (j-block, t-block): exact deps
            qkv = [[big_pool.tile([P, TB], f16, tag=f"qkv{j}_{t}",
                                  name=f"qkv{j}_{t}")
                    for t in range(TT)] for j in range(6)]
            v_sb = big_pool.tile([P, CC, P], f16, tag="v")
            ccs = big_pool.tile([P, T], f16, tag="cc")
            ss2 = big_pool.tile([P, T], f16, tag="ss")
            tri = big_pool.tile([P, P], f16, tag="tri")
            ident = big_pool.tile([P, P], f16, tag="ident")

            # ---- startup DMA, split across the two HWDGE engines and
            # ordered by consumption time.  RoPE swaps are stream_shuffle
            # on DVE (no DMA), so nothing small-urgent ever queues behind
            # this stream.  x[2], x[3] are issued later, from inside the
            # drain schedule, to keep startup HBM bandwidth for what the
            # first 30us actually needs.
            nc.sync.dma_start(w_sb[:, 0, 0:2], wT_d[0, :, 0:2])
            nc.scalar.dma_start(x_sb[:, 0, 0, 0, :], xT_d[0, :, 0, 0, :])
            nc.sync.dma_start(w_sb[:, 0, 2:6], wT_d[0, :, 2:6])
            nc.scalar.dma_start(x_sb[:, 0, 0, 1, :], xT_d[0, :, 0, 1, :])
            nc.sync.dma_start(w_sb[:, 1], wT_d[1])
            nc.scalar.dma_start(x_sb[:, 0, 0, 2, :], xT_d[0, :, 0, 2, :])
            nc.scalar.dma_start(x_sb[:, 0, 0, 3, :], xT_d[0, :, 0, 3, :])
            nc.scalar.dma_start(x_sb[:, 0, 1, :, :], xT_d[0, :, 1])
            nc.sync.dma_start(w_sb[:, 2], wT_d[2])
            nc.scalar.dma_start(x_sb[:, 0, 2, :, :], xT_d[0, :, 2])
            nc.sync.dma_start(w_sb[:, 3], wT_d[3])
            nc.scalar.dma_start(x_sb[:, 0, 3, :, :], xT_d[0, :, 3])
            nc.scalar.dma_start(x_sb[:, 1], xT_d[1])
            nc.sync.dma_start(ccs[:], cc_d[:])
            nc.sync.dma_start(ss2[:], ss_d[:])
            nc.sync.dma_start(tri[:], tri_d[:])
            nc.sync.dma_start(ident[:], id_d[:])

            # exp table prewarm: Act loads the Exp table set now (one-time
            # ~2.7us), well before the first real exp ~35us in.  Emitted
            # after the DMA issues so it doesn't delay the x stream.
            warm = warm_pool.tile([P, TB], f16, tag="warm")
            warm2 = warm_pool.tile([P, 8], f16, tag="warm2")
            nc.vector.memset(warm[:], 0.0)
            nc.scalar.activation(warm2[:], warm[:, 0:8],
                                 mybir.ActivationFunctionType.Exp, scale=1.0)

            # PE p-state warmup: the PE idles ~5us waiting for the first
            # x piece, and silicon ramps from ~2.7x to 1x cycle time over
            # ~3us of continuous execution.  Burn the wait on dummy
            # matmuls so the real projections start at full speed.
            ps_w = pp_pool.tile([P, TB], f32, tag="pp", name="ps_warm")
            for _ in range(14):
                nc.tensor.matmul(ps_w[:], warm[:, 0:P], warm[:],
                                 start=True, stop=True)



            # even/odd head-dim components sit 16 partitions apart within
            # each 32-partition quadrant (host perm), so the RoPE partner
            # swap is a single DVE stream_shuffle -- no DMA.
            SWAP16 = [(i + 16) % 32 for i in range(32)]

            def rope(j, tt):
                """In-place rotate-half RoPE on qkv[j][tt]."""
                tsl = slice(tt * TB, (tt + 1) * TB)
                q = qkv[j][tt]
                swp = swp_pool.tile([P, TB], f16, tag="swp", name="swp")
                nc.vector.stream_shuffle(swp[:], q[:], SWAP16)
                ta = rt_pool.tile([P, TB], f16, tag="ta", name="ta")
                tb_ = rt_pool.tile([P, TB], f16, tag="tb", name="tb")
                nc.vector.tensor_tensor(ta[:], q[:], ccs[:, tsl], mult)
                nc.vector.tensor_tensor(tb_[:], swp[:], ss2[:, tsl], mult)
                nc.vector.tensor_tensor(q[:], ta[:], tb_[:], add)

            def vtrans(tt):
                """v^T [d, s] chunks -> v_sb [s, chunk, d] via PE matmul
                with v^T stationary and identity moving."""
                ps = pp_pool.tile([P, TB], f32, tag="pp", name="vtr")
                for i in range(4):
                    nc.tensor.matmul(
                        ps[:, i * P:(i + 1) * P],
                        qkv[5][tt][:, i * P:(i + 1) * P],
                        ident[:],
                        start=True, stop=True,
                    )
                nc.vector.tensor_copy(v_sb[:, 4 * tt:4 * tt + 4, :], ps[:])

            # ---- attention for one (tb, head-pair) with score lookahead ----
            def att_pass(tb, h0, interleave=None, last=False):
                nsc = 4 * (tb + 1)
                depth = 2
                yp = yp_pool.tile([P, 2, TB], f32, tag="yp", name="yp")
                acc = acc_pool.tile([P, 2, TB], f16, tag="acc", name="acc")
                exts = [None] * nsc

                def col0(sc):
                    r = sc - 4 * tb
                    return r * P if r >= 0 else 0

                def scores(sc):
                    c0 = col0(sc)
                    sp = sp_pool.tile([P, 2, TB], f32, tag="sp", name="sp")
                    for k in range(2):
                        nc.tensor.matmul(
                            sp[:, k, c0:],
                            qkv[4][sc // 4][:, (sc % 4) * P:(sc % 4 + 1) * P],
                            qkv[h0 + k][tb][:, c0:],
                            start=True, stop=True,
                        )
                    ex = exp_pool.tile([P, 2, TB], f16, tag="ex", name="ex")
                    nc.scalar.activation(
                        ex[:, :, c0:], sp[:, :, c0:],
                        mybir.ActivationFunctionType.Exp, scale=SCALE)
                    if sc - 4 * tb >= 0:
                        for k in range(2):
                            nc.vector.tensor_tensor(
                                ex[:, k, c0:c0 + P], ex[:, k, c0:c0 + P],
                                tri[:], mult)
                    if sc == 0:
                        nc.vector.tensor_copy(acc[:], ex[:])
                    else:
                        nc.vector.tensor_tensor(
                            acc[:, :, c0:], ex[:, :, c0:], acc[:, :, c0:], add)
                    exts[sc] = ex

                def pv(sc):
                    c0 = col0(sc)
                    for k in range(2):
                        nc.tensor.matmul(
                            yp[:, k, c0:],
                            v_sb[:, sc, :],
                            exts[sc][:, k, c0:],
                            start=(sc == 0), stop=(sc == nsc - 1),
                        )

                for sc in range(nsc):
                    scores(sc)
                    if interleave is not None:
                        interleave(sc)
                    if sc >= depth:
                        pv(sc - depth)
                # acc is complete after the last scores step: drain now
                (nc.scalar if last else nc.sync).dma_start(
                    acc_d[tb * 2 + h0 // 2], acc[:])
                # tail PVs: y cols [0,256) are final after pv(nsc-3), so
                # their cast overlaps the last two PV matmuls.
                ysb = y_pool.tile([P, 2, TB], f16, tag="ysb", name="ysb")
                pv(nsc - 2)
                nc.vector.tensor_copy(ysb[:, :, 0:3 * P], yp[:, :, 0:3 * P])
                pv(nsc - 1)
                nc.vector.tensor_copy(ysb[:, :, 3 * P:], yp[:, :, 3 * P:])
                nc.sync.dma_start(yT_d[tb, :, h0 // 2], ysb[:])

            # ---- schedule ----
            # initial block: all six j-blocks for tt=0, interleaved at
            # (cq, ci) granularity across six PSUM accumulators (sp/yp
            # pools are free until the first attention pass), so each
            # arriving 128KB x piece feeds 6 matmuls (~1.3us of PE work,
            # matching the startup delivery rate).  Copies on Act (idle),
            # ropes on DVE.
            psa = pp_pool.tile([P, TB], f32, tag="pp", name="psa")
            psb = pp_pool.tile([P, TB], f32, tag="pp", name="psb")
            s01 = sp_pool.tile([P, 2, TB], f32, tag="sp", name="s01")
            s23 = sp_pool.tile([P, 2, TB], f32, tag="sp", name="s23")
            accs = [(4, psa[:]), (5, psb[:]), (0, s01[:, 0]), (1, s01[:, 1]),
                    (2, s23[:, 0]), (3, s23[:, 1])]
            for cq in range(4):
                for ci in range(4):
                    cc = cq * 4 + ci
                    for j, acc_ap in accs:
                        nc.tensor.matmul(
                            acc_ap,
                            w_sb[:, cq, JS[j], ci, :],
                            x_sb[:, 0, cq, ci, :],
                            start=(cc == 0),
                            stop=(cc == CC - 1),
                        )
            nc.scalar.copy(qkv[4][0][:], psa[:])
            rope(4, 0)
            nc.scalar.copy(qkv[5][0][:], psb[:])
            vtrans(0)
            nc.scalar.copy(qkv[0][0][:], s01[:, 0])
            rope(0, 0)
            nc.scalar.copy(qkv[1][0][:], s01[:, 1])
            rope(1, 0)
            nc.scalar.copy(qkv[2][0][:], s23[:, 0])
            rope(2, 0)
            nc.scalar.copy(qkv[3][0][:], s23[:, 1])
            rope(3, 0)
            # re-prewarm the exp table after Act's copy burst, before the
            # first real exp of pass (0,0)
            nc.scalar.activation(warm2[:], warm[:, 0:8],
                                 mybir.ActivationFunctionType.Exp, scale=1.0)
            nc.sync.dma_start(x_sb[:, 2], xT_d[2])

            # remaining 20 proj tiles drain inside the attention passes.
            # Order = deadline order: tiles needed by pass p+1 drain during
            # pass p.  One cq-quarter (4 matmuls) or one rope/vtrans/copy
            # block per yield.
            def gen_drain():
                # (j, tt) in deadline order: tiles needed by pass p+1
                # drain during pass p or earlier.  q0,q1 @tt=3 go last:
                # the final pass is (3,0), so their rope work on DVE
                # overlaps the long (3,2) pass.  x[3] bulk DMA issued
                # from in here (sync idle, HWDGE lanes free by then).
                # v tiles drain LAST within each group: their trailing op
                # is vtrans (PE-side), so the rope-critical q tiles get an
                # extra tile of DVE slack before the pass that needs them.
                # tt=3: v goes LAST -- its vtrans output is first consumed
                # by pass (3,2)'s own pv(12) at step 14, so it can drain
                # inside that pass, filling its otherwise-bare tail.
                # tt=3: v goes LAST -- its vtrans output is first consumed
                # by pass (3,2)'s own pv(12) at step 14, so it can drain
                # inside that pass, filling its otherwise-bare tail.
                order = []
                for tt in range(1, TT):
                    if tt < 3:
                        order += [(4, tt), (0, tt), (1, tt), (5, tt),
                                  (2, tt), (3, tt)]
                    else:
                        order += [(4, tt), (2, tt), (3, tt),
                                  (0, tt), (1, tt), (5, tt)]
                for j, tt in order:
                    ps = pp_pool.tile([P, TB], f32, tag="pp", name="ps")
                    for cq in range(4):
                        for ci in range(4):
                            cc = cq * 4 + ci
                            nc.tensor.matmul(
                                ps[:],
                                w_sb[:, cq, JS[j], ci, :],
                                x_sb[:, tt, cq, ci, :],
                                start=(cc == 0),
                                stop=(cc == CC - 1),
                            )
                        yield
                    nc.vector.tensor_copy(qkv[j][tt][:], ps[:])
                    if j == 5:
                        vtrans(tt)
                    else:
                        rope(j, tt)
                    if (j, tt) == (4, 1):
                        nc.sync.dma_start(x_sb[:, 3], xT_d[3])
                    yield

            it = gen_drain()

            # Budgets: yields needed before each subsequent pass starts
            # (every tile is 5 yields: 4 matmul quads + 1 copy/rope).
            # p0 (0,0): k,v @tt=1 (10)       p1 (0,2): q0,q1,q2 @tt=1 (15)
            # p2 (1,0): q3 @tt=1 (5)         p3 (1,2): k..q1 @tt=2 (20)
            # p4 (2,0): q2,q3 @tt=2 (10)     p5 (2,2): kv,q23 @tt=3 (20)
            # p6 (3,2): q0,q1 @tt=3 (10)     p7 (3,0): bare
            passes = [(0, 0, 10), (0, 2, 15), (1, 0, 5), (1, 2, 20),
                      (2, 0, 10), (2, 2, 15), (3, 2, 15), (3, 0, 0)]

            def make_drain(budget, denom):
                done = [0]

                def drain(sc):
                    # front-loaded pacing: finish the budget a few steps
                    # before pass end so trailing rope/copy DVE work
                    # overlaps the last attention steps.
                    target = min(budget, budget * (sc + 1) // denom)
                    while done[0] < target:
                        try:
                            next(it)
                        except StopIteration:
                            return
                        done[0] += 1
                return drain

            for i, (tb, h0, budget) in enumerate(passes):
                # pass (3,2) drains two rope tiles consumed immediately by
                # the final pass plus v_3 (whose vtrans feeds this pass's
                # own pv(12) at step 14): complete the budget by step 12.
                denom = 12 if (tb, h0) == (3, 2) else max(4 * (tb + 1) - 3, 1)
                att_pass(tb, h0, interleave=make_drain(budget, denom),
                         last=(i == len(passes) - 1))
            for _ in it:  # finish any leftover proj work
                pass

    nc.compile()
    _cache["nc"] = nc
    return nc


def _host_prep(x, w_qkv, freqs_cos, freqs_sin):
    """Build per-core input maps (numpy, cheap)."""
    x = np.asarray(x, dtype=np.float32)
    w_qkv = np.asarray(w_qkv, dtype=np.float32)
    freqs_cos = np.asarray(freqs_cos, dtype=np.float32)
    freqs_sin = np.asarray(freqs_sin, dtype=np.float32)

    # partition p holds head-dim component: quadrant q=p//32, r=p%32;
    # pair j = 16q + r%16; even component (2j) for r<16, odd (2j+1) for
    # r>=16.  Even/odd partners are 16 partitions apart within a
    # 32-partition quadrant, so RoPE's partner swap is a DVE
    # stream_shuffle.
    pp_ = np.arange(P)
    pair = 16 * (pp_ // 32) + (pp_ % 32) % 16
    perm = 2 * pair + (pp_ % 32 >= 16)

    # x^T per batch in [tt, p, cq, ci, tb] layout (16KB contiguous per
    # partition per t-block: whole-tt DMAs use one descriptor per row)
    xTs = []
    for b in range(B):
        xt = x[b].T.reshape(4, 4, P, TT, TB).transpose(3, 2, 0, 1, 4)
        xTs.append(np.ascontiguousarray(xt.astype(np.float16)))

    cosT = freqs_cos.T  # [64, T]
    sinT = freqs_sin.T
    sign = np.where(pp_ % 32 >= 16, 1.0, -1.0)[:, None].astype(np.float32)
    CCh = np.ascontiguousarray(cosT[pair].astype(np.float16))
    SS2 = np.ascontiguousarray((sign * sinT[pair]).astype(np.float16))
    tri = np.triu(np.ones((P, P), dtype=np.float16))
    ident = np.eye(P, dtype=np.float16)

    in_maps = []
    for core in range(NCORES):
        b, kv = divmod(core, KV)
        blocks = []
        for r in range(NREP):
            hrow = (kv * NREP + r) * HD
            blocks.append(w_qkv[hrow:hrow + HD][perm])
        blocks.append(w_qkv[H * HD + kv * HD:H * HD + (kv + 1) * HD][perm])
        blocks.append(
            w_qkv[(H + KV) * HD + kv * HD:(H + KV) * HD + (kv + 1) * HD]
        )
        w_shard = np.concatenate(blocks, axis=0)  # [768, C]
        # [cq, p, slot, ci, col]: c = (cq*4+ci)*128+p, col j*128+d,
        # slot order [k, v, q0, q1, q2, q3]
        jmap = [4, 5, 0, 1, 2, 3]
        wT = w_shard.T.reshape(4, 4, P, 6, P).transpose(0, 2, 3, 1, 4)
        wT = wT[:, :, jmap]
        wT = np.ascontiguousarray(wT.astype(np.float16))
        in_maps.append({
            "xT": xTs[b],
            "wT": wT,
            "CC": CCh,
            "SS2": SS2,
            "tri": tri,
            "ident": ident,
        })
    return in_maps


def kernel(x, w_qkv, freqs_cos, freqs_sin):
    nc = _build()
    in_maps = _host_prep(x, w_qkv, freqs_cos, freqs_sin)
    res = run_bass_kernel_spmd(nc, in_maps, list(range(NCORES)), trace=TRACE)
    _cache["last_res"] = res

    y = np.empty((B, T, C), dtype=np.float32)
    for core in range(NCORES):
        b, kv = divmod(core, KV)
        # [tb, d, pair, head, t] -> [d, head(NREP), t]
        yT = res.results[core]["yT"].astype(np.float32)
        yT = yT.transpose(1, 2, 3, 0, 4).reshape(P, NREP, T)
        accs = res.results[core]["acc"].astype(np.float32)  # [8, P, 2*TB]
        acc = accs.reshape(TT, 2, P, 2, TB)  # [tb, pass, lane, hh, t]
        den = acc.sum(axis=2)  # [tb, pass, hh, t]
        den = den.transpose(1, 2, 0, 3).reshape(NREP, T)  # [h, t]
        y_norm = yT / den[None, :, :]  # [d, h, t]
        y[b, :, kv * NREP * HD:(kv + 1) * NREP * HD] = (
            y_norm.transpose(2, 1, 0).reshape(T, NREP * HD)
        )
    return y
